# revision 14
# baseline (speedup 1.0000x reference)
"""HMM forward algorithm (diagonal-Gaussian emissions) on Trainium2.

Strategy
--------
Data-parallel: batch dim (16) sharded across 8 NeuronCores (2 sequences per
core); the small [N,N] transition / [N,D] emission params are replicated.

Per core, the sequential scan is run in the *linear* domain:
    w_t = (P^T w_{t-1}) * Ehat_t,   Ehat_t = exp(emis_t - max_n emis_t)
with P = softmax(transition_logits) loaded once as stationary PE weights, so
each scan step is one tiny TensorE matmul + one VectorE multiply.  Every
K_RENORM steps w is renormalized by its sum (computed for free via an
appended ones-column in the weights) to stay inside f32 range; the log of
every renorm factor and every per-step max is recovered afterwards with a
bulk triangular-matmul cumulative sum, so
    log_prob[t, n] = C_t + ln(v_t[n]) + ehat_t[n]
which is exact and underflow-proof (v = P^T w never underflows; the deep
tails live in ehat which is kept in log domain).
"""

import numpy as np
from contextlib import ExitStack

import concourse.bacc as bacc
import concourse.bass as bass
import concourse.mybir as mybir
import concourse.tile as tile
from concourse.bass_utils import run_bass_kernel_spmd

F32 = mybir.dt.float32
AX = mybir.AxisListType
ALU = mybir.AluOpType
ACT = mybir.ActivationFunctionType

B, S, N, D = 16, 1000, 64, 80
NCORES = 8
BLOC = B // NCORES          # sequences per core
NCHUNK = 8                  # time chunks for bulk phases
CHUNK = S // NCHUNK         # 125
NCOL = BLOC * NCHUNK        # columns of the (chunk, batch) bookkeeping tiles
LOG_2PI = float(np.log(2.0 * np.pi))


def emit_kernel(ctx: ExitStack, tc: tile.TileContext, aps: dict, k_renorm: int):
    nc = tc.nc
    obs, tl, mu, lv, il = aps["obs"], aps["tl"], aps["mu"], aps["lv"], aps["il"]
    ident_d, triu_d, strib_d = aps["ident"], aps["triu"], aps["strib"]
    usel_d = aps["usel"]
    logp, total = aps["logp"], aps["total"]

    persist = ctx.enter_context(tc.tile_pool(name="persist", bufs=1))
    work = ctx.enter_context(tc.tile_pool(name="work", bufs=3))
    outw = ctx.enter_context(tc.tile_pool(name="outw", bufs=3))
    ps_em = ctx.enter_context(tc.tile_pool(name="ps_em", bufs=2, space="PSUM"))
    ps_tr = ctx.enter_context(tc.tile_pool(name="ps_tr", bufs=2, space="PSUM"))
    ps_v = ctx.enter_context(tc.tile_pool(name="ps_v", bufs=2, space="PSUM"))
    ps_m = ctx.enter_context(tc.tile_pool(name="ps_m", bufs=2, space="PSUM"))

    def sbuf(name, shape):
        return persist.tile(shape, F32, tag=name, name=name)

    # ---- constants from host ----
    ident = sbuf("ident", [128, 128])
    nc.sync.dma_start(ident[:], ident_d[:])
    triu = sbuf("triu", [CHUNK, CHUNK])
    nc.sync.dma_start(triu[:], triu_d[:])
    strib = sbuf("strib", [NCOL, NCOL])
    nc.sync.dma_start(strib[:], strib_d[:])
    ones = sbuf("ones", [128, 128])
    nc.any.memset(ones[:], 1.0)
    usel = sbuf("usel", list(usel_d.shape))
    nc.sync.dma_start(usel[:], usel_d[:])

    def transpose(dst_sb, src, k):
        """dst_sb[f, p] = src[p, f] via PE; src [p<=128, f<=128] SBUF."""
        p, f = src.shape
        t_ps = ps_tr.tile([128, 128], F32, tag="tr", name="tr")
        nc.tensor.matmul(t_ps[:f, :p], src, ident[:p, :p], is_transpose=True)
        nc.any.tensor_copy(dst_sb, t_ps[:f, :p])

    # ---- transition matrix: P_aug[i, j] = softmax(tl)[i, j]; col N = 1 ----
    tl_sb = sbuf("tl_sb", [N, N])
    nc.sync.dma_start(tl_sb[:], tl[:])
    p_aug = sbuf("p_aug", [N, N + 1])
    ngm_t = sbuf("ngm_t", [N, 1])
    z_t = sbuf("z_t", [N, 1])
    rz_t = sbuf("rz_t", [N, 1])
    exl = sbuf("exl", [N, N])
    nc.vector.tensor_reduce(ngm_t[:], tl_sb[:], axis=AX.X, op=ALU.max, negate=True)
    nc.scalar.activation(exl[:], tl_sb[:], ACT.Exp, bias=ngm_t[:], accum_out=z_t[:])
    nc.vector.reciprocal(rz_t[:], z_t[:])
    nc.vector.tensor_scalar_mul(p_aug[:, 0:N], exl[:], rz_t[:])
    nc.any.memset(p_aug[:, N:N + 1], 1.0)

    # ---- initial distribution p0 (probabilities, column vector) ----
    il_sb = sbuf("il_sb", [1, N])
    nc.sync.dma_start(il_sb[:], il.rearrange("(a n) -> a n", a=1))
    ngm_i = sbuf("ngm_i", [1, 1])
    z_i = sbuf("z_i", [1, 1])
    rz_i = sbuf("rz_i", [1, 1])
    exi = sbuf("exi", [1, N])
    p0_row = sbuf("p0_row", [1, N])
    p0 = sbuf("p0", [N, 1])
    nc.vector.tensor_reduce(ngm_i[:], il_sb[:], axis=AX.X, op=ALU.max, negate=True)
    nc.scalar.activation(exi[:], il_sb[:], ACT.Exp, bias=ngm_i[:], accum_out=z_i[:])
    nc.vector.reciprocal(rz_i[:], z_i[:])
    nc.vector.tensor_scalar_mul(p0_row[:], exi[:], rz_i[:])
    transpose(p0[:], p0_row[:], None)

    # ---- emission weights ----
    # emis[t,n] = cp[n] + sum_d obs2[t,d]*W2[d,n] + sum_d obs[t,d]*W1[d,n]
    # W2 = -0.5*exp(-lv)^T, W1 = (mu*exp(-lv))^T,
    # cp[n] = -0.5*(sum_d lv + D*log2pi + sum_d mu^2*exp(-lv))
    lv_sb = sbuf("lv_sb", [N, D])
    mu_sb = sbuf("mu_sb", [N, D])
    nc.sync.dma_start(lv_sb[:], lv[:])
    nc.sync.dma_start(mu_sb[:], mu[:])
    lvt = sbuf("lvt", [D, N])
    mut = sbuf("mut", [D, N])
    transpose(lvt[:], lv_sb[:], None)
    transpose(mut[:], mu_sb[:], None)
    ivt = sbuf("ivt", [D, N])
    w2 = sbuf("w2", [D, N])
    w1 = sbuf("w1", [D, N])
    nc.scalar.activation(ivt[:], lvt[:], ACT.Exp, scale=-1.0)
    nc.vector.tensor_scalar_mul(w2[:], ivt[:], -0.5)
    nc.vector.tensor_mul(w1[:], mut[:], ivt[:])
    iv_nat = sbuf("iv_nat", [N, D])
    mu2 = sbuf("mu2", [N, D])
    m2iv = sbuf("m2iv", [N, D])
    lvs = sbuf("lvs", [N, 1])
    m2s = sbuf("m2s", [N, 1])
    cps = sbuf("cps", [N, 1])
    cp_col = sbuf("cp_col", [N, 1])
    cp_row = sbuf("cp_row", [1, N])
    nc.scalar.activation(iv_nat[:], lv_sb[:], ACT.Exp, scale=-1.0)
    nc.scalar.activation(mu2[:], mu_sb[:], ACT.Square)
    nc.vector.tensor_mul(m2iv[:], mu2[:], iv_nat[:])
    nc.vector.tensor_reduce(lvs[:], lv_sb[:], axis=AX.X, op=ALU.add)
    nc.vector.tensor_reduce(m2s[:], m2iv[:], axis=AX.X, op=ALU.add)
    nc.vector.tensor_add(cps[:], lvs[:], m2s[:])
    # cp = -0.5*(lvs + m2s) - 0.5*D*log(2*pi)
    nc.vector.tensor_scalar(cp_col[:], cps[:], -0.5, -0.5 * D * LOG_2PI,
                            op0=ALU.mult, op1=ALU.add)
    transpose(cp_row[:], cp_col[:], None)

    # ---- per-(b, chunk) emission pass ----
    # E_all / V_all / W_all: [N, BLOC*S] with column index b*S + t
    e_all = sbuf("e_all", [N, BLOC * S])
    v_all = sbuf("v_all", [N, BLOC * S])
    w_all = sbuf("w_all", [N, BLOC * S])
    ehat = sbuf("ehat", [CHUNK, NCOL * N])     # [tp, (b c n)]
    ngm_all = sbuf("ngm_all", [CHUNK, NCOL])   # -max_n emis, [tp, (b c)]
    e3 = e_all[:].rearrange("p (b t) -> p b t", b=BLOC)
    v3 = v_all[:].rearrange("p (b t) -> p b t", b=BLOC)
    w3 = w_all[:].rearrange("p (b t) -> p b t", b=BLOC)
    ehat4 = ehat[:].rearrange("p (b c n) -> p b c n", b=BLOC, c=NCHUNK)

    for b in range(BLOC):
        for c in range(NCHUNK):
            col = c * BLOC + b
            t0 = c * CHUNK
            onat = work.tile([CHUNK, D], F32, tag="onat", name="onat")
            nc.sync.dma_start(onat[:], obs[b, t0:t0 + CHUNK, :])
            ot = work.tile([D, CHUNK], F32, tag="ot", name="ot")
            transpose(ot[:], onat[:], None)
            o2t = work.tile([D, CHUNK], F32, tag="o2t", name="o2t")
            nc.scalar.activation(o2t[:], ot[:], ACT.Square)
            em_ps = ps_em.tile([CHUNK, N], F32, tag="em", name="em")
            nc.tensor.matmul(em_ps[:], o2t[:], w2[:], start=True, stop=False)
            nc.tensor.matmul(em_ps[:], ot[:], w1[:], start=False, stop=False)
            nc.tensor.matmul(em_ps[:], ones[0:1, 0:CHUNK], cp_row[:],
                             start=False, stop=True)
            nc.vector.tensor_reduce(ngm_all[:, col:col + 1], em_ps[:],
                                    axis=AX.X, op=ALU.max, negate=True)
            # ehat = emis - max  (Identity: out = in*1 + bias)
            nc.scalar.activation(ehat4[:, b, c, :], em_ps[:], ACT.Identity,
                                 bias=ngm_all[:, col:col + 1])
            et = work.tile([CHUNK, N], F32, tag="et", name="et")
            nc.scalar.activation(et[:], ehat4[:, b, c, :], ACT.Exp)
            transpose(e3[:, b, t0:t0 + CHUNK], et[:], None)

    # ---- scan ----
    # renorm reciprocals, compact row: col = b * NREN + q  (q = t/K - 1)
    nren = (S - 1) // k_renorm
    r_row = sbuf("r_row", [1, BLOC * nren])
    r3 = r_row[:].rearrange("p (b q) -> p b q", b=BLOC)

    # t = 0: w_0 = p0 * Ehat_0 ; v_0 := p0 (so the output formula is uniform)
    nc.vector.tensor_scalar_mul(w3[:, :, 0], e3[:, :, 0], p0[:])
    for b in range(BLOC):
        nc.scalar.copy(v3[:, b, 0:1], p0[:])

    for t in range(1, S):
        v_ps = ps_v.tile([N + 1, BLOC], F32, tag="v", name="v")
        nc.tensor.matmul(v_ps[:], p_aug[:], w3[:, :, t - 1], start=True, stop=True)
        # persist v for the output reconstruction (ScalarE, off critical path)
        nc.scalar.copy(v3[:, :, t], v_ps[0:N, :])
        if t % k_renorm == 0:
            q = t // k_renorm - 1
            nc.vector.reciprocal(r3[:, :, q], v_ps[N:N + 1, :])
            bc_ps = ps_m.tile([N, BLOC], F32, tag="m", name="m")
            nc.tensor.matmul(bc_ps[:], ones[0:1, 0:N], r3[:, :, q],
                             start=True, stop=True)
            tmp = work.tile([N, BLOC], F32, tag="rtmp", name="rtmp")
            nc.vector.tensor_mul(tmp[:], v_ps[0:N, :], e3[:, :, t])
            nc.vector.tensor_mul(w3[:, :, t], tmp[:], bc_ps[:])
        else:
            nc.vector.tensor_mul(w3[:, :, t], v_ps[0:N, :], e3[:, :, t])

    # ---- reconstruct C'_t = cumsum(mx) + sum of log s from *strictly earlier*
    # renorms, via triangular + selection matmuls ----
    mx_all = sbuf("mx_all", [CHUNK, NCOL])
    nc.vector.tensor_scalar_mul(mx_all[:], ngm_all[:], -1.0)
    c_ps = ps_m.tile([CHUNK, NCOL], F32, tag="m", name="m")
    nc.tensor.matmul(c_ps[:], triu[:], mx_all[:], start=True, stop=False)
    # per-chunk totals of mx (for cross-chunk offsets): ones-column matmul
    tot_ps = ps_m.tile([1, NCOL], F32, tag="m", name="m")
    nc.tensor.matmul(tot_ps[:], ones[0:CHUNK, 0:1], mx_all[:],
                     start=True, stop=True)
    tot_row = sbuf("tot_row", [1, NCOL])
    nc.any.tensor_copy(tot_row[:], tot_ps[:])
    totc = sbuf("totc", [NCOL, 1])
    transpose(totc[:], tot_row[:], None)
    offmat = sbuf("offmat", [NCOL, NCOL])
    nc.vector.tensor_scalar_mul(offmat[:], strib[:], totc[:])
    nc.tensor.matmul(c_ps[:], ones[0:NCOL, 0:CHUNK], offmat[:],
                     start=False, stop=False)
    # renorm corrections: lnR = ln(1/s); C' -= sum_{earlier q} lnR[q]
    lnr = sbuf("lnr", [1, BLOC * nren])
    nc.scalar.activation(lnr[:], r_row[:], ACT.Ln)
    nqb = (nren + 127) // 128
    nlt = sbuf("nlt", [128, nqb * BLOC])   # -lnR, transposed: [q, (blk b)]
    lnr3 = lnr[:].rearrange("p (b q) -> p b q", b=BLOC)
    for b in range(BLOC):
        for blk in range(nqb):
            q0 = blk * 128
            qn = min(128, nren - q0)
            t_ps = ps_tr.tile([128, 128], F32, tag="tr", name="tr")
            nc.tensor.matmul(t_ps[:qn, 0:1], lnr3[:, b, q0:q0 + qn],
                             ident[0:1, 0:1], is_transpose=True)
            nc.scalar.activation(nlt[0:qn, blk * BLOC + b:blk * BLOC + b + 1],
                                 t_ps[:qn, 0:1], ACT.Identity, scale=-1.0)
    for c in range(NCHUNK):
        for blk in range(nqb):
            q0 = blk * 128
            qn = min(128, nren - q0)
            last = (c == NCHUNK - 1) and (blk == nqb - 1)
            nc.tensor.matmul(
                c_ps[:, c * BLOC:(c + 1) * BLOC],
                usel[0:qn, (blk * NCHUNK + c) * CHUNK:(blk * NCHUNK + c + 1) * CHUNK],
                nlt[0:qn, blk * BLOC:(blk + 1) * BLOC],
                start=False, stop=last)
    c_out = sbuf("c_out", [CHUNK, NCOL])
    nc.any.tensor_copy(c_out[:], c_ps[:])

    # ---- outputs: logp[b, t, n] = C'_t + ln(v_t[n]) + ehat_t[n] ----
    for b in range(BLOC):
        for c in range(NCHUNK):
            col = c * BLOC + b
            t0 = c * CHUNK
            vt_ps = ps_tr.tile([128, 128], F32, tag="tr", name="tr")
            nc.tensor.matmul(vt_ps[:CHUNK, :N], v3[:, b, t0:t0 + CHUNK],
                             ident[0:N, 0:N], is_transpose=True)
            lnv = outw.tile([CHUNK, N], F32, tag="lnv", name="lnv")
            nc.scalar.activation(lnv[:], vt_ps[:CHUNK, :N], ACT.Ln)
            s1 = outw.tile([CHUNK, N], F32, tag="s1", name="s1")
            nc.vector.tensor_add(s1[:], lnv[:], ehat4[:, b, c, :])
            fin = outw.tile([CHUNK, N], F32, tag="fin", name="fin")
            nc.vector.tensor_scalar_add(fin[:], s1[:], c_out[:, col:col + 1])
            nc.sync.dma_start(logp[b, t0:t0 + CHUNK, :], fin[:])

    # ---- total_log_prob = C_T(inclusive) + ln(sum_n w_T)
    #      C_T = sum_t mx_t + sum_q log s_q = reduce(tot_row) - reduce(lnR) ----
    sf_ps = ps_m.tile([1, BLOC], F32, tag="m", name="m")
    nc.tensor.matmul(sf_ps[:], ones[0:N, 0:1], w3[:, :, S - 1], start=True, stop=True)
    lsf = sbuf("lsf", [1, BLOC])
    nc.scalar.activation(lsf[:], sf_ps[:], ACT.Ln)
    redt = sbuf("redt", [1, BLOC])
    nc.vector.tensor_reduce(redt[:], tot_row[:].rearrange("p (c b) -> p b c", b=BLOC),
                            axis=AX.X, op=ALU.add)
    redl = sbuf("redl", [1, BLOC])
    nc.vector.tensor_reduce(redl[:], lnr3, axis=AX.X, op=ALU.add)
    t1 = sbuf("t1", [1, BLOC])
    nc.vector.tensor_add(t1[:], lsf[:], redt[:])
    tot_sb = sbuf("tot_sb", [1, BLOC])
    nc.vector.tensor_sub(tot_sb[:], t1[:], redl[:])
    nc.sync.dma_start(total.rearrange("(a b) -> a b", a=1), tot_sb[:])


def geom(k_renorm: int):
    nren = (S - 1) // k_renorm
    nqb = max(1, (nren + 127) // 128)
    return nren, nqb


def build_nc(k_renorm: int) -> bass.Bass:
    nren, nqb = geom(k_renorm)
    nc = bacc.Bacc("TRN2", target_bir_lowering=False, debug=False)
    aps = {
        "obs": nc.dram_tensor("observations", [BLOC, S, D], F32,
                              kind="ExternalInput").ap(),
        "tl": nc.dram_tensor("transition_logits", [N, N], F32,
                             kind="ExternalInput").ap(),
        "mu": nc.dram_tensor("emission_means", [N, D], F32,
                             kind="ExternalInput").ap(),
        "lv": nc.dram_tensor("emission_logvars", [N, D], F32,
                             kind="ExternalInput").ap(),
        "il": nc.dram_tensor("initial_logits", [N], F32,
                             kind="ExternalInput").ap(),
        "ident": nc.dram_tensor("ident", [128, 128], F32,
                                kind="ExternalInput").ap(),
        "triu": nc.dram_tensor("triu", [CHUNK, CHUNK], F32,
                               kind="ExternalInput").ap(),
        "strib": nc.dram_tensor("strib", [NCOL, NCOL], F32,
                                kind="ExternalInput").ap(),
        "usel": nc.dram_tensor("usel", [128, nqb * NCHUNK * CHUNK], F32,
                               kind="ExternalInput").ap(),
        "logp": nc.dram_tensor("logp", [BLOC, S, N], F32,
                               kind="ExternalOutput").ap(),
        "total": nc.dram_tensor("total", [BLOC], F32,
                                kind="ExternalOutput").ap(),
    }
    with tile.TileContext(nc) as tc:
        with ExitStack() as ctx:
            emit_kernel(ctx, tc, aps, k_renorm)
    nc.compile()
    return nc


def host_constants(k_renorm: int):
    nren, nqb = geom(k_renorm)
    ident = np.eye(128, dtype=np.float32)
    # triu[t', t] = 1 iff t' <= t  (inclusive cumsum over the chunk)
    triu = np.triu(np.ones((CHUNK, CHUNK), dtype=np.float32))
    # (c, b)-major columns: strib[c1*BLOC+b, c2*BLOC+b] = 1 iff c1 < c2
    strib = np.zeros((NCOL, NCOL), dtype=np.float32)
    for bb in range(BLOC):
        for c2 in range(NCHUNK):
            for c1 in range(c2):
                strib[c1 * BLOC + bb, c2 * BLOC + bb] = 1.0
    # usel[q_in_blk, (blk*NCHUNK + c)*CHUNK + tp] = 1 iff renorm q happened
    # strictly before t = c*CHUNK + tp, i.e. k*(q+1) < c*CHUNK + tp
    usel = np.zeros((128, nqb * NCHUNK * CHUNK), dtype=np.float32)
    for blk in range(nqb):
        for qi in range(min(128, nren - blk * 128)):
            q = blk * 128 + qi
            tq = k_renorm * (q + 1)
            for c in range(NCHUNK):
                for tp in range(CHUNK):
                    if tq < c * CHUNK + tp:
                        usel[qi, (blk * NCHUNK + c) * CHUNK + tp] = 1.0
    return ident, triu, strib, usel


def pick_k_renorm(transition_logits: np.ndarray) -> int:
    x = transition_logits.astype(np.float64)
    mx = x.max(-1, keepdims=True)
    lse = np.log(np.sum(np.exp(x - mx), -1, keepdims=True)) + mx
    logp = x - lse
    min_lp = float(logp.min())
    # window decay is bounded below by (min_ij P_ij)^K; keep it above e^-66
    k = int(66.0 // max(1.0, -min_lp))
    return max(1, min(16, k))


def kernel(observations, transition_logits, emission_means, emission_logvars,
           initial_logits):
    observations = np.asarray(observations, dtype=np.float32)
    transition_logits = np.asarray(transition_logits, dtype=np.float32)
    emission_means = np.asarray(emission_means, dtype=np.float32)
    emission_logvars = np.asarray(emission_logvars, dtype=np.float32)
    initial_logits = np.asarray(initial_logits, dtype=np.float32)

    k_renorm = pick_k_renorm(transition_logits)
    nc = build_nc(k_renorm)
    ident, triu, strib, usel = host_constants(k_renorm)

    in_maps = []
    for c in range(NCORES):
        in_maps.append({
            "observations": observations[c * BLOC:(c + 1) * BLOC],
            "transition_logits": transition_logits,
            "emission_means": emission_means,
            "emission_logvars": emission_logvars,
            "initial_logits": initial_logits,
            "ident": ident,
            "triu": triu,
            "strib": strib,
            "usel": usel,
        })
    res = run_bass_kernel_spmd(nc, in_maps, list(range(NCORES)))
    logp = np.concatenate([res.results[c]["logp"] for c in range(NCORES)], axis=0)
    total = np.concatenate([res.results[c]["total"] for c in range(NCORES)], axis=0)
    return logp, total


# revision 16
# speedup vs baseline: 1.3279x; 1.3279x over previous
"""HMM forward algorithm (diagonal-Gaussian emissions) on Trainium2.

Strategy
--------
Data-parallel: batch dim (16) sharded across 8 NeuronCores (2 sequences per
core); the small [N,N] transition / [N,D] emission params are replicated.

Per core, the sequential scan is run in the *linear* domain:
    w_t = (P^T w_{t-1}) * Ehat_t,   Ehat_t = exp(emis_t - max_n emis_t)
with P = softmax(transition_logits) loaded once as stationary PE weights, so
each scan step is one tiny TensorE matmul + one VectorE multiply.  Every
K_RENORM steps w is renormalized by its sum (computed for free via an
appended ones-column in the weights) to stay inside f32 range; the log of
every renorm factor and every per-step max is recovered afterwards with a
bulk triangular-matmul cumulative sum, so
    log_prob[t, n] = C_t + ln(v_t[n]) + ehat_t[n]
which is exact and underflow-proof (v = P^T w never underflows; the deep
tails live in ehat which is kept in log domain).
"""

import numpy as np
from contextlib import ExitStack

import concourse.bacc as bacc
import concourse.bass as bass
import concourse.mybir as mybir
import concourse.tile as tile
from concourse.bass_utils import run_bass_kernel_spmd

F32 = mybir.dt.float32
AX = mybir.AxisListType
ALU = mybir.AluOpType
ACT = mybir.ActivationFunctionType

B, S, N, D = 16, 1000, 64, 80
NCORES = 8
BLOC = B // NCORES          # sequences per core
NCHUNK = 8                  # time chunks for bulk phases
CHUNK = S // NCHUNK         # 125
NCOL = BLOC * NCHUNK        # columns of the (chunk, batch) bookkeeping tiles
LOG_2PI = float(np.log(2.0 * np.pi))


def emit_kernel(ctx: ExitStack, tc: tile.TileContext, aps: dict, k_renorm: int):
    nc = tc.nc
    obs, tl, mu, lv, il = aps["obs"], aps["tl"], aps["mu"], aps["lv"], aps["il"]
    ident_d, triu_d, strib_d = aps["ident"], aps["triu"], aps["strib"]
    usel_d = aps["usel"]
    logp, total = aps["logp"], aps["total"]

    persist = ctx.enter_context(tc.tile_pool(name="persist", bufs=1))
    work = ctx.enter_context(tc.tile_pool(name="work", bufs=3))
    outw = ctx.enter_context(tc.tile_pool(name="outw", bufs=3))
    ps_em = ctx.enter_context(tc.tile_pool(name="ps_em", bufs=2, space="PSUM"))
    ps_tr = ctx.enter_context(tc.tile_pool(name="ps_tr", bufs=2, space="PSUM"))
    ps_v = ctx.enter_context(tc.tile_pool(name="ps_v", bufs=2, space="PSUM"))
    ps_m = ctx.enter_context(tc.tile_pool(name="ps_m", bufs=2, space="PSUM"))

    def sbuf(name, shape):
        return persist.tile(shape, F32, tag=name, name=name)

    # ---- constants from host ----
    ident = sbuf("ident", [128, 128])
    nc.sync.dma_start(ident[:], ident_d[:])
    triu = sbuf("triu", [CHUNK, CHUNK])
    nc.sync.dma_start(triu[:], triu_d[:])
    strib = sbuf("strib", [NCOL, NCOL])
    nc.sync.dma_start(strib[:], strib_d[:])
    ones = sbuf("ones", [128, 128])
    nc.any.memset(ones[:], 1.0)
    usel = sbuf("usel", list(usel_d.shape))
    nc.sync.dma_start(usel[:], usel_d[:])

    def transpose(dst_sb, src, k):
        """dst_sb[f, p] = src[p, f] via PE; src [p<=128, f<=128] SBUF."""
        p, f = src.shape
        t_ps = ps_tr.tile([128, 128], F32, tag="tr", name="tr")
        nc.tensor.matmul(t_ps[:f, :p], src, ident[:p, :p], is_transpose=True)
        nc.any.tensor_copy(dst_sb, t_ps[:f, :p])

    # ---- transition matrix: P_aug[i, j] = softmax(tl)[i, j]; col N = 1 ----
    tl_sb = sbuf("tl_sb", [N, N])
    nc.sync.dma_start(tl_sb[:], tl[:])
    p_aug = sbuf("p_aug", [N, N + 1])
    ngm_t = sbuf("ngm_t", [N, 1])
    z_t = sbuf("z_t", [N, 1])
    rz_t = sbuf("rz_t", [N, 1])
    exl = sbuf("exl", [N, N])
    nc.vector.tensor_reduce(ngm_t[:], tl_sb[:], axis=AX.X, op=ALU.max, negate=True)
    nc.scalar.activation(exl[:], tl_sb[:], ACT.Exp, bias=ngm_t[:], accum_out=z_t[:])
    nc.vector.reciprocal(rz_t[:], z_t[:])
    nc.vector.tensor_scalar_mul(p_aug[:, 0:N], exl[:], rz_t[:])
    nc.any.memset(p_aug[:, N:N + 1], 1.0)

    # ---- initial distribution p0 (probabilities, column vector) ----
    il_sb = sbuf("il_sb", [1, N])
    nc.sync.dma_start(il_sb[:], il.rearrange("(a n) -> a n", a=1))
    ngm_i = sbuf("ngm_i", [1, 1])
    z_i = sbuf("z_i", [1, 1])
    rz_i = sbuf("rz_i", [1, 1])
    exi = sbuf("exi", [1, N])
    p0_row = sbuf("p0_row", [1, N])
    p0 = sbuf("p0", [N, 1])
    nc.vector.tensor_reduce(ngm_i[:], il_sb[:], axis=AX.X, op=ALU.max, negate=True)
    nc.scalar.activation(exi[:], il_sb[:], ACT.Exp, bias=ngm_i[:], accum_out=z_i[:])
    nc.vector.reciprocal(rz_i[:], z_i[:])
    nc.vector.tensor_scalar_mul(p0_row[:], exi[:], rz_i[:])
    transpose(p0[:], p0_row[:], None)

    # ---- emission weights ----
    # emis[t,n] = cp[n] + sum_d obs2[t,d]*W2[d,n] + sum_d obs[t,d]*W1[d,n]
    # W2 = -0.5*exp(-lv)^T, W1 = (mu*exp(-lv))^T,
    # cp[n] = -0.5*(sum_d lv + D*log2pi + sum_d mu^2*exp(-lv))
    lv_sb = sbuf("lv_sb", [N, D])
    mu_sb = sbuf("mu_sb", [N, D])
    nc.sync.dma_start(lv_sb[:], lv[:])
    nc.sync.dma_start(mu_sb[:], mu[:])
    lvt = sbuf("lvt", [D, N])
    mut = sbuf("mut", [D, N])
    transpose(lvt[:], lv_sb[:], None)
    transpose(mut[:], mu_sb[:], None)
    ivt = sbuf("ivt", [D, N])
    w2 = sbuf("w2", [D, N])
    w1 = sbuf("w1", [D, N])
    nc.scalar.activation(ivt[:], lvt[:], ACT.Exp, scale=-1.0)
    nc.vector.tensor_scalar_mul(w2[:], ivt[:], -0.5)
    nc.vector.tensor_mul(w1[:], mut[:], ivt[:])
    iv_nat = sbuf("iv_nat", [N, D])
    mu2 = sbuf("mu2", [N, D])
    m2iv = sbuf("m2iv", [N, D])
    lvs = sbuf("lvs", [N, 1])
    m2s = sbuf("m2s", [N, 1])
    cps = sbuf("cps", [N, 1])
    cp_col = sbuf("cp_col", [N, 1])
    cp_row = sbuf("cp_row", [1, N])
    nc.scalar.activation(iv_nat[:], lv_sb[:], ACT.Exp, scale=-1.0)
    nc.scalar.activation(mu2[:], mu_sb[:], ACT.Square)
    nc.vector.tensor_mul(m2iv[:], mu2[:], iv_nat[:])
    nc.vector.tensor_reduce(lvs[:], lv_sb[:], axis=AX.X, op=ALU.add)
    nc.vector.tensor_reduce(m2s[:], m2iv[:], axis=AX.X, op=ALU.add)
    nc.vector.tensor_add(cps[:], lvs[:], m2s[:])
    # cp = -0.5*(lvs + m2s) - 0.5*D*log(2*pi)
    nc.vector.tensor_scalar(cp_col[:], cps[:], -0.5, -0.5 * D * LOG_2PI,
                            op0=ALU.mult, op1=ALU.add)
    transpose(cp_row[:], cp_col[:], None)

    # ---- per-(b, chunk) emission pass ----
    # E_all / V_all / W_all: [N, BLOC*S] with column index b*S + t
    e_all = sbuf("e_all", [N, BLOC * S])
    v_all = sbuf("v_all", [N, BLOC * S])
    w_all = sbuf("w_all", [N, BLOC * S])
    ehat = sbuf("ehat", [CHUNK, NCOL * N])     # [tp, (b c n)]
    ngm_all = sbuf("ngm_all", [CHUNK, NCOL])   # -max_n emis, [tp, (b c)]
    e3 = e_all[:].rearrange("p (b t) -> p b t", b=BLOC)
    v3 = v_all[:].rearrange("p (b t) -> p b t", b=BLOC)
    w3 = w_all[:].rearrange("p (b t) -> p b t", b=BLOC)
    ehat4 = ehat[:].rearrange("p (b c n) -> p b c n", b=BLOC, c=NCHUNK)

    for b in range(BLOC):
        for c in range(NCHUNK):
            col = c * BLOC + b
            t0 = c * CHUNK
            onat = work.tile([CHUNK, D], F32, tag="onat", name="onat")
            nc.sync.dma_start(onat[:], obs[b, t0:t0 + CHUNK, :])
            ot = work.tile([D, CHUNK], F32, tag="ot", name="ot")
            transpose(ot[:], onat[:], None)
            o2t = work.tile([D, CHUNK], F32, tag="o2t", name="o2t")
            nc.scalar.activation(o2t[:], ot[:], ACT.Square)
            em_ps = ps_em.tile([CHUNK, N], F32, tag="em", name="em")
            nc.tensor.matmul(em_ps[:], o2t[:], w2[:], start=True, stop=False)
            nc.tensor.matmul(em_ps[:], ot[:], w1[:], start=False, stop=False)
            nc.tensor.matmul(em_ps[:], ones[0:1, 0:CHUNK], cp_row[:],
                             start=False, stop=True)
            nc.vector.tensor_reduce(ngm_all[:, col:col + 1], em_ps[:],
                                    axis=AX.X, op=ALU.max, negate=True)
            # ehat = emis - max  (Identity: out = in*1 + bias)
            nc.scalar.activation(ehat4[:, b, c, :], em_ps[:], ACT.Identity,
                                 bias=ngm_all[:, col:col + 1])
            et = work.tile([CHUNK, N], F32, tag="et", name="et")
            nc.scalar.activation(et[:], ehat4[:, b, c, :], ACT.Exp)
            transpose(e3[:, b, t0:t0 + CHUNK], et[:], None)

    # ---- scan ----
    # renorm reciprocals, compact row: col = b * NREN + q  (q = t/K - 1)
    nren = (S - 1) // k_renorm
    r_row = sbuf("r_row", [1, BLOC * nren])
    r3 = r_row[:].rearrange("p (b q) -> p b q", b=BLOC)

    # t = 0: w_0 = p0 * Ehat_0 ; v_0 := p0 (so the output formula is uniform)
    nc.vector.tensor_scalar_mul(w3[:, :, 0], e3[:, :, 0], p0[:])
    for b in range(BLOC):
        nc.scalar.copy(v3[:, b, 0:1], p0[:])

    for t in range(1, S):
        v_ps = ps_v.tile([N + 1, BLOC], F32, tag="v", name="v")
        nc.tensor.matmul(v_ps[:], p_aug[:], w3[:, :, t - 1], start=True, stop=True)
        if t % k_renorm == 0:
            q = t // k_renorm - 1
            nc.vector.reciprocal(r3[:, :, q], v_ps[N:N + 1, :])
            bc_ps = ps_m.tile([N, BLOC], F32, tag="m", name="m")
            nc.tensor.matmul(bc_ps[:], ones[0:1, 0:N], r3[:, :, q],
                             start=True, stop=True)
            tmp = work.tile([N, BLOC], F32, tag="rtmp", name="rtmp")
            nc.vector.tensor_mul(tmp[:], v_ps[0:N, :], e3[:, :, t])
            nc.vector.tensor_mul(w3[:, :, t], tmp[:], bc_ps[:])
        else:
            nc.vector.tensor_mul(w3[:, :, t], v_ps[0:N, :], e3[:, :, t])

    # ---- bulk-recompute v_t = P^T w_{t-1} for the output reconstruction ----
    for b in range(BLOC):
        t0 = 0
        while t0 < S - 1:
            tn = min(500, S - 1 - t0)
            vb_ps = ps_em.tile([N, 500], F32, tag="em", name="em")
            nc.tensor.matmul(vb_ps[:, 0:tn], p_aug[:, 0:N], w3[:, b, t0:t0 + tn],
                             start=True, stop=True)
            nc.scalar.copy(v3[:, b, t0 + 1:t0 + 1 + tn], vb_ps[:, 0:tn])
            t0 += tn

    # ---- reconstruct C'_t = cumsum(mx) + sum of log s from *strictly earlier*
    # renorms, via triangular + selection matmuls ----
    mx_all = sbuf("mx_all", [CHUNK, NCOL])
    nc.vector.tensor_scalar_mul(mx_all[:], ngm_all[:], -1.0)
    c_ps = ps_m.tile([CHUNK, NCOL], F32, tag="m", name="m")
    nc.tensor.matmul(c_ps[:], triu[:], mx_all[:], start=True, stop=False)
    # per-chunk totals of mx (for cross-chunk offsets): ones-column matmul
    tot_ps = ps_m.tile([1, NCOL], F32, tag="m", name="m")
    nc.tensor.matmul(tot_ps[:], ones[0:CHUNK, 0:1], mx_all[:],
                     start=True, stop=True)
    tot_row = sbuf("tot_row", [1, NCOL])
    nc.any.tensor_copy(tot_row[:], tot_ps[:])
    totc = sbuf("totc", [NCOL, 1])
    transpose(totc[:], tot_row[:], None)
    offmat = sbuf("offmat", [NCOL, NCOL])
    nc.vector.tensor_scalar_mul(offmat[:], strib[:], totc[:])
    nc.tensor.matmul(c_ps[:], ones[0:NCOL, 0:CHUNK], offmat[:],
                     start=False, stop=False)
    # renorm corrections: lnR = ln(1/s); C' -= sum_{earlier q} lnR[q]
    lnr = sbuf("lnr", [1, BLOC * nren])
    nc.scalar.activation(lnr[:], r_row[:], ACT.Ln)
    nqb = (nren + 127) // 128
    nlt = sbuf("nlt", [128, nqb * BLOC])   # -lnR, transposed: [q, (blk b)]
    lnr3 = lnr[:].rearrange("p (b q) -> p b q", b=BLOC)
    for b in range(BLOC):
        for blk in range(nqb):
            q0 = blk * 128
            qn = min(128, nren - q0)
            t_ps = ps_tr.tile([128, 128], F32, tag="tr", name="tr")
            nc.tensor.matmul(t_ps[:qn, 0:1], lnr3[:, b, q0:q0 + qn],
                             ident[0:1, 0:1], is_transpose=True)
            nc.scalar.activation(nlt[0:qn, blk * BLOC + b:blk * BLOC + b + 1],
                                 t_ps[:qn, 0:1], ACT.Identity, scale=-1.0)
    for c in range(NCHUNK):
        for blk in range(nqb):
            q0 = blk * 128
            qn = min(128, nren - q0)
            last = (c == NCHUNK - 1) and (blk == nqb - 1)
            nc.tensor.matmul(
                c_ps[:, c * BLOC:(c + 1) * BLOC],
                usel[0:qn, (blk * NCHUNK + c) * CHUNK:(blk * NCHUNK + c + 1) * CHUNK],
                nlt[0:qn, blk * BLOC:(blk + 1) * BLOC],
                start=False, stop=last)
    c_out = sbuf("c_out", [CHUNK, NCOL])
    nc.any.tensor_copy(c_out[:], c_ps[:])

    # ---- outputs: logp[b, t, n] = C'_t + ln(v_t[n]) + ehat_t[n] ----
    for b in range(BLOC):
        for c in range(NCHUNK):
            col = c * BLOC + b
            t0 = c * CHUNK
            vt_ps = ps_tr.tile([128, 128], F32, tag="tr", name="tr")
            nc.tensor.matmul(vt_ps[:CHUNK, :N], v3[:, b, t0:t0 + CHUNK],
                             ident[0:N, 0:N], is_transpose=True)
            lnv = outw.tile([CHUNK, N], F32, tag="lnv", name="lnv")
            nc.scalar.activation(lnv[:], vt_ps[:CHUNK, :N], ACT.Ln)
            s1 = outw.tile([CHUNK, N], F32, tag="s1", name="s1")
            nc.vector.tensor_add(s1[:], lnv[:], ehat4[:, b, c, :])
            fin = outw.tile([CHUNK, N], F32, tag="fin", name="fin")
            nc.vector.tensor_scalar_add(fin[:], s1[:], c_out[:, col:col + 1])
            nc.sync.dma_start(logp[b, t0:t0 + CHUNK, :], fin[:])

    # ---- total_log_prob = C_T(inclusive) + ln(sum_n w_T)
    #      C_T = sum_t mx_t + sum_q log s_q = reduce(tot_row) - reduce(lnR) ----
    sf_ps = ps_m.tile([1, BLOC], F32, tag="m", name="m")
    nc.tensor.matmul(sf_ps[:], ones[0:N, 0:1], w3[:, :, S - 1], start=True, stop=True)
    lsf = sbuf("lsf", [1, BLOC])
    nc.scalar.activation(lsf[:], sf_ps[:], ACT.Ln)
    redt = sbuf("redt", [1, BLOC])
    nc.vector.tensor_reduce(redt[:], tot_row[:].rearrange("p (c b) -> p b c", b=BLOC),
                            axis=AX.X, op=ALU.add)
    redl = sbuf("redl", [1, BLOC])
    nc.vector.tensor_reduce(redl[:], lnr3, axis=AX.X, op=ALU.add)
    t1 = sbuf("t1", [1, BLOC])
    nc.vector.tensor_add(t1[:], lsf[:], redt[:])
    tot_sb = sbuf("tot_sb", [1, BLOC])
    nc.vector.tensor_sub(tot_sb[:], t1[:], redl[:])
    nc.sync.dma_start(total.rearrange("(a b) -> a b", a=1), tot_sb[:])


def geom(k_renorm: int):
    nren = (S - 1) // k_renorm
    nqb = max(1, (nren + 127) // 128)
    return nren, nqb


def build_nc(k_renorm: int) -> bass.Bass:
    nren, nqb = geom(k_renorm)
    nc = bacc.Bacc("TRN2", target_bir_lowering=False, debug=False)
    aps = {
        "obs": nc.dram_tensor("observations", [BLOC, S, D], F32,
                              kind="ExternalInput").ap(),
        "tl": nc.dram_tensor("transition_logits", [N, N], F32,
                             kind="ExternalInput").ap(),
        "mu": nc.dram_tensor("emission_means", [N, D], F32,
                             kind="ExternalInput").ap(),
        "lv": nc.dram_tensor("emission_logvars", [N, D], F32,
                             kind="ExternalInput").ap(),
        "il": nc.dram_tensor("initial_logits", [N], F32,
                             kind="ExternalInput").ap(),
        "ident": nc.dram_tensor("ident", [128, 128], F32,
                                kind="ExternalInput").ap(),
        "triu": nc.dram_tensor("triu", [CHUNK, CHUNK], F32,
                               kind="ExternalInput").ap(),
        "strib": nc.dram_tensor("strib", [NCOL, NCOL], F32,
                                kind="ExternalInput").ap(),
        "usel": nc.dram_tensor("usel", [128, nqb * NCHUNK * CHUNK], F32,
                               kind="ExternalInput").ap(),
        "logp": nc.dram_tensor("logp", [BLOC, S, N], F32,
                               kind="ExternalOutput").ap(),
        "total": nc.dram_tensor("total", [BLOC], F32,
                                kind="ExternalOutput").ap(),
    }
    with tile.TileContext(nc) as tc:
        with ExitStack() as ctx:
            emit_kernel(ctx, tc, aps, k_renorm)
    nc.compile()
    return nc


def host_constants(k_renorm: int):
    nren, nqb = geom(k_renorm)
    ident = np.eye(128, dtype=np.float32)
    # triu[t', t] = 1 iff t' <= t  (inclusive cumsum over the chunk)
    triu = np.triu(np.ones((CHUNK, CHUNK), dtype=np.float32))
    # (c, b)-major columns: strib[c1*BLOC+b, c2*BLOC+b] = 1 iff c1 < c2
    strib = np.zeros((NCOL, NCOL), dtype=np.float32)
    for bb in range(BLOC):
        for c2 in range(NCHUNK):
            for c1 in range(c2):
                strib[c1 * BLOC + bb, c2 * BLOC + bb] = 1.0
    # usel[q_in_blk, (blk*NCHUNK + c)*CHUNK + tp] = 1 iff renorm q happened
    # strictly before t = c*CHUNK + tp, i.e. k*(q+1) < c*CHUNK + tp
    usel = np.zeros((128, nqb * NCHUNK * CHUNK), dtype=np.float32)
    for blk in range(nqb):
        for qi in range(min(128, nren - blk * 128)):
            q = blk * 128 + qi
            tq = k_renorm * (q + 1)
            for c in range(NCHUNK):
                for tp in range(CHUNK):
                    if tq < c * CHUNK + tp:
                        usel[qi, (blk * NCHUNK + c) * CHUNK + tp] = 1.0
    return ident, triu, strib, usel


def pick_k_renorm(transition_logits: np.ndarray) -> int:
    x = transition_logits.astype(np.float64)
    mx = x.max(-1, keepdims=True)
    lse = np.log(np.sum(np.exp(x - mx), -1, keepdims=True)) + mx
    logp = x - lse
    min_lp = float(logp.min())
    # window decay is bounded below by (min_ij P_ij)^K; keep it above e^-66
    k = int(66.0 // max(1.0, -min_lp))
    return max(1, min(16, k))


def kernel(observations, transition_logits, emission_means, emission_logvars,
           initial_logits):
    observations = np.asarray(observations, dtype=np.float32)
    transition_logits = np.asarray(transition_logits, dtype=np.float32)
    emission_means = np.asarray(emission_means, dtype=np.float32)
    emission_logvars = np.asarray(emission_logvars, dtype=np.float32)
    initial_logits = np.asarray(initial_logits, dtype=np.float32)

    k_renorm = pick_k_renorm(transition_logits)
    nc = build_nc(k_renorm)
    ident, triu, strib, usel = host_constants(k_renorm)

    in_maps = []
    for c in range(NCORES):
        in_maps.append({
            "observations": observations[c * BLOC:(c + 1) * BLOC],
            "transition_logits": transition_logits,
            "emission_means": emission_means,
            "emission_logvars": emission_logvars,
            "initial_logits": initial_logits,
            "ident": ident,
            "triu": triu,
            "strib": strib,
            "usel": usel,
        })
    res = run_bass_kernel_spmd(nc, in_maps, list(range(NCORES)))
    logp = np.concatenate([res.results[c]["logp"] for c in range(NCORES)], axis=0)
    total = np.concatenate([res.results[c]["total"] for c in range(NCORES)], axis=0)
    return logp, total
